# revision 5
# baseline (speedup 1.0000x reference)
"""Trainium2 Bass kernel for CustomLossWithCovariance.

loss = abs(logdet(sigma) + mean_b[(p_b - t_b)^T sigma^{-1} (p_b - t_b)])

Only the 3x3 Gram matrix G = sum_b d_b d_b^T (d = pred - targ) requires
touching the [B, 3] data; the device computes per-core partial pair-sums
of G, and the host finishes with the tiny 3x3 algebra:
    mean_mahalanobis = <sigma_inv, G> / B
    loss = |logdet(sigma) + mean_mahalanobis|

Sharding: data-parallel over the batch across 8 NeuronCores (each core
streams a contiguous [B/8, 3] shard; partial sums gathered on host).

Per-core device kernel (raw Bacc, manual semaphores — see
build_gram_kernel_raw; build_gram_kernel is the Tile-framework
baseline kept for reference). Per tile of [128, 2M]:
  - DMA both halves (pred | targ) flat-contiguous in one dma_start
  - DVE: d = pred - targ, in-place into the pred half (unit-stride fp32)
  - ACT: Square(d_i) with accum_out -> per-partition sums of d_i^2
  - DVE: scalar_tensor_tensor(d_i * d_j, accum_out) -> cross sums
  (component APs are stride-3 views of the flat tiles, grouped 4 tiles
  per reduce instruction to amortize fixed costs)
"""

import numpy as np

import concourse.bass as bass
import concourse.bacc as bacc
import concourse.mybir as mybir
from concourse import tile
from concourse.bass_utils import run_bass_kernel_spmd

N_CORES = 8
B_FULL = 8388608
P = 128

_PAIRS = [(0, 1), (0, 2), (1, 2)]


def build_gram_kernel(n_rows: int, n_tiles: int, use_act: bool = True):
    """Build the per-core Bass module.

    Input: pt [2, n_rows, 3] f32 (pred stacked with targ)
    Output: partials [128, 6 * n_tiles] f32
        col t*3+i            : sum over this tile/partition of d_i^2
        col 3*n_tiles + t*3+k: sum of d_i*d_j for pair k in _PAIRS
    """
    assert n_rows % (P * n_tiles) == 0
    r = n_rows // (P * n_tiles)  # rows per partition per tile
    m = 3 * r                    # flat f32 elements per partition per tile
    f32 = mybir.dt.float32

    # Bacc (not plain Bass): its compile() pass legalizes semaphore waits
    # (each TRN2 instruction holds at most one wait slot).
    nc = bacc.Bacc("TRN2", target_bir_lowering=False, debug=False)
    pt = nc.dram_tensor("pt", [2, n_rows, 3], f32, kind="ExternalInput")
    out = nc.dram_tensor("partials", [P, 6 * n_tiles], f32, kind="ExternalOutput")

    # [t][p][w(2), m] — per tile/partition: pred chunk and targ chunk, each
    # m contiguous f32 in DRAM.
    pt_v = pt[:].rearrange("w (t p r) c -> t p w (r c)", t=n_tiles, p=P)

    with tile.TileContext(nc) as tc:
        with (
            tc.tile_pool(name="io", bufs=3) as io_pool,
            tc.tile_pool(name="dve_scr", bufs=2) as dve_scr,
            tc.tile_pool(name="act_scr", bufs=2) as act_scr,
            tc.tile_pool(name="acc", bufs=1) as acc_pool,
        ):
            acc_sq = acc_pool.tile([P, 3 * n_tiles], f32)
            acc_cr = acc_pool.tile([P, 3 * n_tiles], f32)

            for t in range(n_tiles):
                buf = io_pool.tile([P, 2 * m], f32, tag="buf")
                nc.sync.dma_start(
                    out=buf[:].rearrange("p (w m) -> p w m", w=2),
                    in_=pt_v[t],
                )

                # In-place: d = pred - targ, overwriting the pred half.
                nc.vector.tensor_tensor(
                    out=buf[:, 0:m],
                    in0=buf[:, 0:m],
                    in1=buf[:, m : 2 * m],
                    op=mybir.AluOpType.subtract,
                )
                d3 = buf[:, 0:m].rearrange("p (r c) -> p c r", c=3)

                # Diagonal sums on the scalar engine (Square + accum_out),
                # overlapping with the DVE cross-products.
                if use_act:
                    for i in range(3):
                        sq = act_scr.tile([P, r], f32, tag="sq")
                        nc.scalar.activation(
                            out=sq[:],
                            in_=d3[:, i, :],
                            func=mybir.ActivationFunctionType.Square,
                            accum_out=acc_sq[:, t * 3 + i : t * 3 + i + 1],
                        )
                else:
                    for i in range(3):
                        sq = dve_scr.tile([P, r], f32, tag="pr")
                        nc.vector.scalar_tensor_tensor(
                            out=sq[:],
                            in0=d3[:, i, :],
                            scalar=1.0,
                            in1=d3[:, i, :],
                            op0=mybir.AluOpType.mult,
                            op1=mybir.AluOpType.mult,
                            accum_out=acc_sq[:, t * 3 + i : t * 3 + i + 1],
                        )
                # Cross sums: fused multiply+reduce on DVE
                # (scalar_tensor_tensor: out = (in0 * 1.0) * in1, accum = sum).
                for k, (i, j) in enumerate(_PAIRS):
                    pr = dve_scr.tile([P, r], f32, tag="pr")
                    nc.vector.scalar_tensor_tensor(
                        out=pr[:],
                        in0=d3[:, i, :],
                        scalar=1.0,
                        in1=d3[:, j, :],
                        op0=mybir.AluOpType.mult,
                        op1=mybir.AluOpType.mult,
                        accum_out=acc_cr[:, t * 3 + k : t * 3 + k + 1],
                    )

            nc.sync.dma_start(out=out[:, 0 : 3 * n_tiles], in_=acc_sq[:])
            nc.sync.dma_start(out=out[:, 3 * n_tiles : 6 * n_tiles], in_=acc_cr[:])

    nc.compile()
    return nc


def build_gram_kernel_raw(n_rows: int, n_tiles: int = 32, n_bufs: int = 24,
                          group: int = 4, skip_exit_barrier: bool = True):
    """Raw-Bacc variant: manual semaphores, no TileContext.

    Skips Tile's prologue/epilogue (drain + two all-engine EVSEM
    barriers, ~16 us) — the only sync needed is a three-semaphore chain:
    DMA loads (one HWDGE ring) -> DVE -> ACT.

    The ring of tile buffers lives in ONE SBUF tensor so the fused
    multiply-reduces can span `group` consecutive tiles with a single
    instruction (free-dim AP [group, r]) — amortizing the per-op fixed
    cost and the accumulator-drain, which keeps both compute engines
    well under the DMA pace.

    Input: pt [2, n_rows, 3] f32. Output: partials [128, 6 * n_groups]
    (same slot layout as build_gram_kernel, with n_groups slots).
    """
    assert n_tiles % group == 0 and n_bufs % group == 0
    assert n_rows % (P * n_tiles) == 0
    n_groups = n_tiles // group
    r = n_rows // (P * n_tiles)
    m = 3 * r
    f32 = mybir.dt.float32

    nc = bacc.Bacc("TRN2", target_bir_lowering=False, debug=False)
    pt = nc.dram_tensor("pt", [2, n_rows, 3], f32, kind="ExternalInput")
    out = nc.dram_tensor("partials", [P, 6 * n_groups], f32, kind="ExternalOutput")
    pt_v = pt[:].rearrange("w (t p r) c -> t p w (r c)", t=n_tiles, p=P)

    ring = nc.alloc_sbuf_tensor("ring", [P, n_bufs * 2 * m], f32).ap()

    def buf(t):
        s = t % n_bufs
        return ring[:, s * 2 * m : (s + 1) * 2 * m]

    def dgroup(g, i):
        # component i of the diff halves of tiles 4g..4g+3: [128, group, r]
        s0 = (g * group) % n_bufs
        w = ring[:, s0 * 2 * m : (s0 + group) * 2 * m]
        return w.rearrange("p (t w r c) -> p t w c r", t=group, w=2, c=3)[:, :, 0, i, :]

    acc_sq = nc.alloc_sbuf_tensor("acc_sq", [P, 3 * n_groups], f32).ap()
    acc_cr = nc.alloc_sbuf_tensor("acc_cr", [P, 3 * n_groups], f32).ap()
    # Rotated scratch (dead stores of the fused ops), 2 groups deep so each
    # group's single stale semaphore wait also covers the scratch WAW from
    # two groups back.
    pr_scrs = [
        nc.alloc_sbuf_tensor(f"pr_scr{k}", [P, group * r], f32).ap() for k in range(6)
    ]
    sq_scrs = [
        nc.alloc_sbuf_tensor(f"sq_scr{k}", [P, group * r], f32).ap() for k in range(6)
    ]

    # One DMA-completion semaphore per ring buffer: a single shared sem
    # would be unsound — each dma_start is split across 16 SDMA engines
    # whose sub-completions interleave across in-flight DMAs.
    dma_sems = [nc.alloc_semaphore(f"dma_sem{i}") for i in range(n_bufs)]
    out_sem = nc.alloc_semaphore("out_sem")
    dve_sem = nc.alloc_semaphore("dve_sem")
    act_sem = nc.alloc_semaphore("act_sem")

    # DVE emission order: subs run ahead; the grouped multiply-reduces for
    # group g are emitted after sub(4g+4) so their drain-wait on the last
    # sub of the group is already satisfied when it executes (DVE writes
    # drain asynchronously). Only the last group trails the final sub.
    dve_order = []
    for t in range(n_tiles):
        dve_order.append(("sub", t))
        if t % group == 0 and t >= group:
            # one sub of stagger after the group's last sub
            dve_order.append(("stt", t // group - 1))
    dve_order.append(("stt", n_groups - 1))
    sub_done, sttg_done = {}, {}
    v = 0
    for kind, x in dve_order:
        if kind == "sub":
            v += 1
            sub_done[x] = v
        else:
            v += 3
            sttg_done[x] = v

    # Output chunks: flush finished accumulator columns while later tiles
    # still stream, so the tail only waits on the last small chunk.
    chunk = max(1, n_groups // 2)
    chunks = [(c, min(c + chunk, n_groups)) for c in range(0, n_groups, chunk)]

    import contextlib

    @contextlib.contextmanager
    def _block():
        # no_gpsimd_drain=True emits per-engine drains explicitly and then a
        # sem-only all-engine butterfly. The butterfly only delays NEFF end
        # (outputs are already fenced by the sequencer's out_sem wait), so
        # optionally no-op it during Block.__exit__.
        with nc.Block(no_gpsimd_drain=True) as blk:
            try:
                yield blk
            finally:
                if skip_exit_barrier:
                    nc.all_engine_barrier = lambda **kw: None
        if skip_exit_barrier:
            del nc.all_engine_barrier  # restore class method

    with _block() as block:

        @block.sync
        def _(sync):
            for t in range(n_tiles):
                if t >= n_bufs:
                    # ring reuse: all consumers of the buffer's previous
                    # occupant (tile t - n_bufs) must be done
                    prev = t - n_bufs
                    sync.wait_ge(dve_sem, sttg_done[prev // group])
                    sync.wait_ge(act_sem, 3 * (prev // group + 1))
                sync.dma_start(
                    out=buf(t).rearrange("p (w m) -> p w m", w=2),
                    in_=pt_v[t],
                ).then_inc(dma_sems[t % n_bufs], 16)
            n_out = 0
            for lo, hi in chunks:
                sync.wait_ge(act_sem, 3 * hi)
                sync.dma_start(
                    out=out[:, 3 * lo : 3 * hi], in_=acc_sq[:, 3 * lo : 3 * hi]
                ).then_inc(out_sem, 16)
                sync.wait_ge(dve_sem, sttg_done[hi - 1])
                sync.dma_start(
                    out=out[:, 3 * (n_groups + lo) : 3 * (n_groups + hi)],
                    in_=acc_cr[:, 3 * lo : 3 * hi],
                ).then_inc(out_sem, 16)
                n_out += 32
            sync.wait_ge(out_sem, n_out)

        @block.vector
        def _(vector):
            for kind, x in dve_order:
                if kind == "sub":
                    b = buf(x)
                    vector.wait_ge(dma_sems[x % n_bufs], 16 * (x // n_bufs + 1))
                    vector.tensor_tensor(
                        out=b[:, 0:m],
                        in0=b[:, 0:m],
                        in1=b[:, m : 2 * m],
                        op=mybir.AluOpType.subtract,
                    ).then_inc(dve_sem, 1)
                else:
                    vector.wait_ge(dve_sem, sub_done[(x + 1) * group - 1])
                    for k, (i, j) in enumerate(_PAIRS):
                        vector.scalar_tensor_tensor(
                            out=pr_scrs[(x % 2) * 3 + k][:].rearrange(
                                "p (t r) -> p t r", t=group
                            ),
                            in0=dgroup(x, i),
                            scalar=1.0,
                            in1=dgroup(x, j),
                            op0=mybir.AluOpType.mult,
                            op1=mybir.AluOpType.mult,
                            accum_out=acc_cr[:, x * 3 + k : x * 3 + k + 1],
                        ).then_inc(dve_sem, 1)

        @block.scalar
        def _(scalar):
            for g in range(n_groups):
                scalar.wait_ge(dve_sem, sub_done[(g + 1) * group - 1])
                if g >= 2:
                    # scratch slot reuse from two groups back
                    scalar.wait_ge(act_sem, 3 * (g - 1))
                for i in range(3):
                    scalar.activation(
                        out=sq_scrs[(g % 2) * 3 + i][:].rearrange(
                            "p (t r) -> p t r", t=group
                        ),
                        in_=dgroup(g, i),
                        func=mybir.ActivationFunctionType.Square,
                        accum_out=acc_sq[:, g * 3 + i : g * 3 + i + 1],
                    ).then_inc(act_sem, 1)

    nc.compile()
    return nc

def _strip_entry_barriers(nc):
    """Remove the two all-engine entry barriers Bass.__init__ emits.

    They serialize ~4us of semaphore round-trips before the first DMA can
    issue. The only cross-engine ordering they provide that this kernel
    needs is gpsimd-const-AP-memset -> ACT-bias-read, which is re-fenced
    explicitly with boot_sem in build_gram_kernel_v3.
    """
    bar = set(nc.barrier_sems)
    blk = nc.main_func.blocks[0]
    drop = []
    for ins in blk.instructions:
        si = getattr(ins, "sync_info", None)
        if si is None:
            continue
        sems = {w.id for w in si.on_wait or []}
        sems |= {u.id for u in si.on_update or []}
        if sems & bar:
            drop.append(ins)
    for ins in drop:
        blk.instructions.remove(ins)
    return len(drop)


def build_gram_kernel_v3(n_rows: int, n_tiles: int = 16, n_bufs: int = 8,
                         group: int = 4, strip_barriers: bool = True,
                         skip_exit_barrier: bool = True):
    """v3: planar-bf16 d + 2x DVE reduces + ACT squares.

    Per tile: DMA both halves -> DVE sub (fp32 in, planar bf16 out:
    component planes x|y|z so reduce operands are unit-stride 2-byte,
    unlocking the DVE 2x perf mode) -> DVE cross-product reduces (grouped
    `group` tiles per instr) + ACT Square reduces (grouped; per-tile for
    the last group so the post-DMA tail stays short).

    The fp32 ring slot is freed by the sub alone (d lives in its own
    full-size buffer), so the DMA stream runs ~n_bufs tiles ahead of
    compute and never stalls on the reduce bursts.

    Output layout [128, 3*n_groups + 3*(n_groups-1) + 3*group]:
      cols 0 .. 3*n_groups-1: cross sums (group g, pair k at 3g+k)
      then squares: full groups 0..n_groups-2 (3 each), then the last
      group's tiles individually (3 each).
    """
    assert n_rows % (P * n_tiles) == 0 and n_tiles % group == 0
    r = n_rows // (P * n_tiles)
    m = 3 * r
    n_groups = n_tiles // group
    full_sq = n_groups - 1           # square-groups emitted grouped
    tail0 = full_sq * group          # first per-tile-squares tile
    ncr = 3 * n_groups
    nsq = 3 * full_sq + 3 * group
    f32, bf16 = mybir.dt.float32, mybir.dt.bfloat16

    nc = bacc.Bacc("TRN2", target_bir_lowering=False, debug=False)
    if strip_barriers:
        _strip_entry_barriers(nc)
    pt = nc.dram_tensor("pt", [2, n_rows, 3], f32, kind="ExternalInput")
    out = nc.dram_tensor("partials", [P, ncr + nsq], f32, kind="ExternalOutput")
    pt_v = pt[:].rearrange("w (t p r) c -> t p w (r c)", t=n_tiles, p=P)

    ring = nc.alloc_sbuf_tensor("ring", [P, n_bufs * 2 * m], f32).ap()
    d_all = nc.alloc_sbuf_tensor("d_all", [P, n_tiles * m], bf16).ap()
    d_t = d_all.rearrange("p (t c r) -> p t c r", t=n_tiles, c=3)
    acc_cr = nc.alloc_sbuf_tensor("acc_cr", [P, ncr], f32).ap()
    acc_sq = nc.alloc_sbuf_tensor("acc_sq", [P, nsq], f32).ap()
    # Dead stores of the fused reduces; single slot per engine (each
    # engine executes its own stream in order, so WAW is safe).
    cr_scr = nc.alloc_sbuf_tensor("cr_scr", [P, group * r], bf16).ap()
    sq_scr = nc.alloc_sbuf_tensor("sq_scr", [P, group * r], bf16).ap()

    dma_sems = [nc.alloc_semaphore(f"dma{i}") for i in range(n_bufs)]
    sub_sem = nc.alloc_semaphore("sub_sem")
    red_sem = nc.alloc_semaphore("red_sem")
    act_sem = nc.alloc_semaphore("act_sem")
    out_sem = nc.alloc_semaphore("out_sem")
    boot_sem = nc.alloc_semaphore("boot_sem")

    def dcomp(t0, nt, i):
        # component i of tiles t0..t0+nt-1: [128, nt, r] unit-stride bf16
        v = d_t[:, t0 : t0 + nt, i, :]
        return v

    import contextlib

    @contextlib.contextmanager
    def _block():
        with nc.Block(no_gpsimd_drain=True) as blk:
            try:
                yield blk
            finally:
                if skip_exit_barrier:
                    nc.all_engine_barrier = lambda **kw: None
        if skip_exit_barrier:
            del nc.all_engine_barrier  # restore class method

    with _block() as block:

        @block.gpsimd
        def _(gpsimd):
            # Const-AP memsets (ACT bias) are earlier in gpsimd's stream;
            # this inc publishes their completion to the scalar queue.
            gpsimd.sem_inc(boot_sem, 1)

        @block.sync
        def _(sync):
            for t in range(n_tiles):
                if t >= n_bufs:
                    # ring slot free once its previous occupant was subbed
                    sync.wait_ge(sub_sem, t - n_bufs + 1)
                sync.dma_start(
                    out=ring[:, (t % n_bufs) * 2 * m : (t % n_bufs + 1) * 2 * m]
                    .rearrange("p (w m) -> p w m", w=2),
                    in_=pt_v[t],
                ).then_inc(dma_sems[t % n_bufs], 16)
            # accumulator flush: big chunks early, last-group slivers at end
            sync.wait_ge(red_sem, 3 * (n_groups - 1))
            sync.dma_start(
                out=out[:, 0 : 3 * (n_groups - 1)],
                in_=acc_cr[:, 0 : 3 * (n_groups - 1)],
            ).then_inc(out_sem, 16)
            sync.wait_ge(act_sem, 3 * full_sq)
            sync.dma_start(
                out=out[:, ncr : ncr + 3 * full_sq],
                in_=acc_sq[:, 0 : 3 * full_sq],
            ).then_inc(out_sem, 16)
            sync.wait_ge(red_sem, 3 * n_groups)
            sync.dma_start(
                out=out[:, 3 * (n_groups - 1) : ncr],
                in_=acc_cr[:, 3 * (n_groups - 1) : ncr],
            ).then_inc(out_sem, 16)
            sync.wait_ge(act_sem, nsq)
            sync.dma_start(
                out=out[:, ncr + 3 * full_sq : ncr + nsq],
                in_=acc_sq[:, 3 * full_sq : nsq],
            ).then_inc(out_sem, 16)
            sync.wait_ge(out_sem, 64)

        @block.vector
        def _(vector):
            for t in range(n_tiles):
                s = t % n_bufs
                buf = ring[:, s * 2 * m : (s + 1) * 2 * m]
                vector.wait_ge(dma_sems[s], 16 * (t // n_bufs + 1))
                # d = pred - targ, downcast to bf16, scattered into
                # component planes (write AP [r, 3] w/ strides [1, r])
                vector.tensor_tensor(
                    out=d_all[:, t * m : (t + 1) * m].rearrange(
                        "p (c r) -> p r c", c=3
                    ),
                    in0=buf[:, 0:m],
                    in1=buf[:, m : 2 * m],
                    op=mybir.AluOpType.subtract,
                ).then_inc(sub_sem, 1)
                if t % group == group - 1:
                    g = t // group
                    for k, (i, j) in enumerate(_PAIRS):
                        vector.scalar_tensor_tensor(
                            out=cr_scr[:].rearrange("p (t r) -> p t r", t=group),
                            in0=dcomp(g * group, group, i),
                            scalar=1.0,
                            in1=dcomp(g * group, group, j),
                            op0=mybir.AluOpType.mult,
                            op1=mybir.AluOpType.mult,
                            accum_out=acc_cr[:, g * 3 + k : g * 3 + k + 1],
                        ).then_inc(red_sem, 1)

        @block.scalar
        def _(scalar):
            scalar.wait_ge(boot_sem, 1)
            for g in range(full_sq):
                scalar.wait_ge(sub_sem, group * (g + 1))
                for i in range(3):
                    scalar.activation(
                        out=sq_scr[:].rearrange("p (t r) -> p t r", t=group),
                        in_=dcomp(g * group, group, i),
                        func=mybir.ActivationFunctionType.Square,
                        accum_out=acc_sq[:, g * 3 + i : g * 3 + i + 1],
                    ).then_inc(act_sem, 1)
            for w, t in enumerate(range(tail0, n_tiles)):
                scalar.wait_ge(sub_sem, t + 1)
                for i in range(3):
                    c = 3 * full_sq + w * 3 + i
                    scalar.activation(
                        out=sq_scr[:, 0:r],
                        in_=dcomp(t, 1, i),
                        func=mybir.ActivationFunctionType.Square,
                        accum_out=acc_sq[:, c : c + 1],
                    ).then_inc(act_sem, 1)

    nc.compile()
    nc._v3_meta = (n_tiles, group)
    return nc


def gram_from_partials_v3(partials: np.ndarray, n_tiles: int, group: int) -> np.ndarray:
    n_groups = n_tiles // group
    ncr = 3 * n_groups
    s = partials.astype(np.float64).reshape(-1, partials.shape[-1]).sum(axis=0)
    cr = s[:ncr].reshape(-1, 3).sum(axis=0)
    sq = s[ncr:].reshape(-1, 3).sum(axis=0)
    g = np.empty((3, 3), dtype=np.float64)
    g[0, 0], g[1, 1], g[2, 2] = sq
    for k, (i, j) in enumerate(_PAIRS):
        g[i, j] = g[j, i] = cr[k]
    return g


_NC_CACHE: dict[tuple, object] = {}


def _get_nc(n_rows: int, n_tiles: int, use_act: bool, raw: bool = False,
            group: int = 4, version: int = 3, n_bufs: int = 8,
            strip_barriers: bool = True):
    key = (n_rows, n_tiles, use_act, raw, group, version, n_bufs, strip_barriers)
    if key not in _NC_CACHE:
        if version == 3:
            _NC_CACHE[key] = build_gram_kernel_v3(
                n_rows, n_tiles, n_bufs=n_bufs, group=group,
                strip_barriers=strip_barriers)
        elif raw:
            _NC_CACHE[key] = build_gram_kernel_raw(n_rows, n_tiles, group=group)
        else:
            _NC_CACHE[key] = build_gram_kernel(n_rows, n_tiles, use_act)
    return _NC_CACHE[key]


def gram_from_partials(partials: np.ndarray, n_tiles: int | None = None) -> np.ndarray:
    """[..., 128, 6*slots] partials -> full 3x3 Gram matrix (float64)."""
    slots = partials.shape[-1] // 6
    s = partials.astype(np.float64).reshape(-1, 6 * slots).sum(axis=0)
    sq = s[: 3 * slots].reshape(slots, 3).sum(axis=0)
    cr = s[3 * slots :].reshape(slots, 3).sum(axis=0)
    g = np.empty((3, 3), dtype=np.float64)
    g[0, 0], g[1, 1], g[2, 2] = sq
    for k, (i, j) in enumerate(_PAIRS):
        g[i, j] = g[j, i] = cr[k]
    return g


def run_device_partials(predictions: np.ndarray, targets: np.ndarray,
                        n_tiles: int = 4, use_act: bool = True,
                        raw: bool = False, group: int = 4, version: int = 3,
                        n_bufs: int = 8, strip_barriers: bool = True,
                        **run_kwargs):
    """Shard over N_CORES, run on device, return per-core partials + results."""
    b = predictions.shape[0]
    assert b % N_CORES == 0
    n_rows = b // N_CORES
    nc = _get_nc(n_rows, n_tiles, use_act, raw, group, version, n_bufs,
                 strip_barriers)
    preds = np.ascontiguousarray(predictions, dtype=np.float32).reshape(
        N_CORES, n_rows, 3
    )
    targs = np.ascontiguousarray(targets, dtype=np.float32).reshape(
        N_CORES, n_rows, 3
    )
    in_maps = [
        {"pt": np.stack([preds[c], targs[c]])} for c in range(N_CORES)
    ]
    res = run_bass_kernel_spmd(nc, in_maps, list(range(N_CORES)), **run_kwargs)
    partials = np.stack([r["partials"] for r in res.results])
    return partials, res


def kernel(predictions: np.ndarray, targets: np.ndarray, sigma: np.ndarray) -> np.ndarray:
    predictions = np.asarray(predictions, dtype=np.float32)
    targets = np.asarray(targets, dtype=np.float32)
    sigma64 = np.asarray(sigma, dtype=np.float64)

    n_tiles, group = 16, 4
    partials, _ = run_device_partials(
        predictions, targets, n_tiles=n_tiles, group=group, version=3)
    g = gram_from_partials_v3(partials, n_tiles, group)

    sigma_inv = np.linalg.inv(sigma64)
    _, logdet = np.linalg.slogdet(sigma64)
    mean_mahal = float((sigma_inv * g).sum()) / predictions.shape[0]
    loss = abs(logdet + mean_mahal)
    return np.float32(loss)

